# revision 39
# baseline (speedup 1.0000x reference)
"""Trainium2 Bass kernel for the Centroid (segment_reduce) problem.

new_centroid = 0.3 * (segment_sum(embed, y) / counts) + 0.7 * centroid
  embed [32768, 1024] f32, y [32768] int (0..999), centroid [1000, 1024] f32

Strategy (8 NeuronCores, CLASS-parallel via host-side routing):
  - The host partitions the 1000 classes into 8 groups of <=128 classes,
    balanced by sample count (LPT + swap refinement; for the uniform
    label distribution every group lands at ~4096 of the 32768 samples).
  - Core i receives ONLY the embed rows whose label falls in its group
    (as fp8 e4m3, padded with zero rows to a fixed CAP).  Each core
    fully owns its classes so there is NO collective at all.
  - On device the scatter-add is a one-hot matmul on TensorE (fp8,
    DoubleRow over k-tile pairs) with a SINGLE 128-slot class tile:
        sums[slot, d] = sum_b onehot[b, slot] * embed[b, d]
    The EMA blend rides the same accumulation: a bf16 exact-diagonal
    matmul adds qcent = 0.7*centroid/scale into PSUM mid-stream, where
    scale = 0.3/count is computed host-side (quantized to 7 mantissa
    bits) and shipped inside the qcent tensor; the single final
    per-slot multiply by scale yields 0.3*mean + 0.7*centroid.
  - The host scatters slot rows back to class rows.

v9 is RAW bass (no TileContext): every engine's instruction stream and
every semaphore is explicit (same-engine read-after-write needs an
explicit completion sync -- the engines are pipelined and do not
interlock).  The embed DMAs are the first instructions in the program;
slot labels for all k-tiles ride inside the first embed tile (bitcast
on device) so nothing gates on a separate label transfer, and the
first tile is a single k-tile so the PE starts as early as possible.
Queue loads are sized to each queue's measured share of DMA service
(SWDGE emits 4KB packets vs HWDGE's 2KB, so it gets ~2x the per-packet
round-robin share; the aggregate is capped by the 16 SDMA engines at
~21 B/ns each).  A burst of dummy matmuls over a zeroed tile warms the
PE's HAM clock-gate to 2.4 GHz before the first real tile lands.  The
epilogue is three scale-multiplies (DVE takes the big chunk whose
accumulation stops first, ACT the other two) and three output slices
on the two HWDGE queues.
"""

import numpy as np

import concourse.bacc as bacc
import concourse.mybir as mybir
from concourse.bass_utils import run_bass_kernel_spmd

N_CORES = 8
C = 1000  # real classes
D = 1024  # embed dim
W = 1024  # embed row bytes (no extra columns)
W0 = 1168  # first-tile width: W + 128B of f32 slot labels + 16B pad
WC = 1040  # qcent row: D bf16 cols + scale (f32 as 2 bf16 cols) + pad
B = 32768  # total batch
P = 128
FACTOR = 0.3
# matmul column chunks (PSUM bank limit is 512 f32); biggest chunk is
# computed FIRST per group so its stop fires earliest and its (longest)
# epilogue scale overlaps the last matmuls
CHUNKS = [(0, 512), (512, 384), (896, 128)]

_F32 = mybir.dt.float32
_BF16 = mybir.dt.bfloat16
_FP8 = mybir.dt.float8e4

_CACHE: dict = {}


def _build(cap: int):
    kt = cap // P  # k-tiles per core (32)
    kh = kt // 2  # k-tiles per class-half (16)
    # half A: kt0 solo (labels) + 7 pairs + kt15 solo; half B: 8 pairs.
    # Half A's accumulation stops mid-stream, so its epilogue and output
    # overlap half B's stream instead of riding the tail.
    ng = 17

    nc = bacc.Bacc(
        "TRN2", target_bir_lowering=False, debug=False, num_devices=N_CORES
    )
    # emb0[p, :] = k-tile 0; cols 1024:1152 hold f32 slot labels for all
    # kt k-tiles (label of padded row k*128+p; pads hold 1.0 which can
    # never equal the iota values 256..383)
    emb0 = nc.dram_tensor("emb0", [P, W0], _FP8, kind="ExternalInput").ap()
    embr = nc.dram_tensor("embr", [P, kt - 1, W], _FP8, kind="ExternalInput").ap()
    # cent[p, 0:1024] = qcent = 0.7*centroid/scale (bf16);
    # cols 1024:1026 = scale (f32 bitcast, low 16 bits zeroed)
    cent = nc.dram_tensor("cent", [P, WC], _BF16, kind="ExternalInput").ap()
    out = nc.dram_tensor("out", [P, D], _BF16, kind="ExternalOutput").ap()

    # SBUF / PSUM
    t0 = nc.alloc_sbuf_tensor("t0", [P, W0], _FP8)
    embt = {
        j: nc.alloc_sbuf_tensor(f"e{j}", [P, 2, W], _FP8)
        for j in range(1, 16)
    }
    t15 = nc.alloc_sbuf_tensor("t15", [P, 1, W], _FP8)
    oh0 = nc.alloc_sbuf_tensor("oh0", [P, P], _FP8)
    ohp = [
        nc.alloc_sbuf_tensor(f"oh{j}", [P, 2, P], _FP8) for j in range(1, 16)
    ]
    ohl = nc.alloc_sbuf_tensor("ohl", [P, P], _FP8)
    iota = nc.alloc_sbuf_tensor("iota", [P, P], _F32)
    iotac = nc.alloc_sbuf_tensor("iotac", [P, 1], _F32)
    diag = nc.alloc_sbuf_tensor("diag", [P, P], _BF16)
    centb = nc.alloc_sbuf_tensor("centb", [P, WC], _BF16)
    outb = nc.alloc_sbuf_tensor("outb", [P, D], _BF16)
    dummy = nc.alloc_sbuf_tensor("warm_sb", [P, 640], _FP8)
    psA = [
        nc.alloc_psum_tensor(f"psA{q}", [P, n], _F32)
        for q, (_, n) in enumerate(CHUNKS)
    ]
    ps = [
        nc.alloc_psum_tensor(f"ps{q}", [P, n], _F32)
        for q, (_, n) in enumerate(CHUNKS)
    ]
    ps_warm = nc.alloc_psum_tensor("ps_warm", [P, 512], _F32)
    scale_ap = centb[:, D : D + 2].bitcast(_F32)

    # semaphores (one per DMA transfer, plus pipeline edges)
    sE = {g: nc.alloc_semaphore(f"sE{g}") for g in range(ng)}
    sC = nc.alloc_semaphore("sC")
    sIo = nc.alloc_semaphore("sIo")
    sOH = nc.alloc_semaphore("sOH")
    sDg = nc.alloc_semaphore("sDg")
    sM = [nc.alloc_semaphore(f"sM{q}") for q in range(3)]
    sMA = [nc.alloc_semaphore(f"sMA{q}") for q in range(3)]
    sVA = nc.alloc_semaphore("sVA")
    sAA = nc.alloc_semaphore("sAA")
    sV1 = nc.alloc_semaphore("sV1")
    sV2 = nc.alloc_semaphore("sV2")
    sA0 = nc.alloc_semaphore("sA0")
    sA2 = nc.alloc_semaphore("sA2")
    sZ = nc.alloc_semaphore("sZ")
    sOutA = nc.alloc_semaphore("sOutA")
    sOutB = nc.alloc_semaphore("sOutB")

    def pair_src(j):
        # A-pairs j=1..7: k-tiles 2j-1,2j; B-pairs j=8..15: k-tiles 2j,2j+1
        lo = 2 * j - 2 if j <= 7 else 2 * j - 1
        return embr[:, lo : lo + 2, :]

    # --- sync queue: tiny first tile (gates everything), then its embed
    # share, qcent, and the solo last k-tile ---
    nc.sync.dma_start(out=t0[:], in_=emb0).then_inc(sE[0], 16)
    nc.sync.dma_start(out=embt[4][:], in_=pair_src(4)).then_inc(sE[4], 16)
    nc.sync.dma_start(out=centb[:], in_=cent).then_inc(sC, 16)
    nc.sync.dma_start(out=t15[:], in_=embr[:, 14:15, :]).then_inc(sE[16], 16)
    nc.sync.dma_start(out=embt[8][:], in_=pair_src(8)).then_inc(sE[8], 16)
    nc.sync.dma_start(out=embt[12][:], in_=pair_src(12)).then_inc(sE[12], 16)

    # --- scalar queue ---
    for j in (2, 6, 10, 14):
        nc.scalar.dma_start(out=embt[j][:], in_=pair_src(j)).then_inc(sE[j], 16)

    # --- gpsimd queue (SWDGE; sustains ~1.6x a HWDGE queue's share, so
    # it carries every other pair; its DMAs go out before the warm-up
    # memset so the queue starts streaming as early as possible) ---
    nc.gpsimd.memset(dummy[:], 0).then_inc(sZ)
    for j in (1, 3):
        nc.gpsimd.dma_start(out=embt[j][:], in_=pair_src(j)).then_inc(sE[j], 16)
    nc.gpsimd.iota(
        iota[:],
        pattern=[[1, P]],
        base=256,
        channel_multiplier=0,
        allow_small_or_imprecise_dtypes=True,
    ).then_inc(sIo)
    nc.gpsimd.iota(
        iotac[:],
        pattern=[[1, 1]],
        base=256,
        channel_multiplier=1,
        allow_small_or_imprecise_dtypes=True,
    ).then_inc(sIo)
    for j in (5, 7, 9, 11, 13, 15):
        nc.gpsimd.dma_start(out=embt[j][:], in_=pair_src(j)).then_inc(sE[j], 16)

    # --- vector: one-hot builds (labels all live in t0) + diagonal ---
    def ysl(k):
        return t0[:, W + 4 * k : W + 4 * k + 4].bitcast(_F32)

    nc.vector.wait_ge(sIo, 2)
    nc.vector.wait_ge(sE[0], 16)
    nc.vector.tensor_scalar(
        oh0[:], iota[:], ysl(0), None, mybir.AluOpType.is_equal
    ).then_inc(sOH)
    def build_pair(j):
        for j2 in range(2):
            k = (2 * j - 1 + j2) if j <= 7 else (2 * j + j2)
            ins = nc.vector.tensor_scalar(
                ohp[j - 1][:, j2, :],
                iota[:],
                ysl(k),
                None,
                mybir.AluOpType.is_equal,
            )
            if j2 == 1:
                ins.then_inc(sOH)

    for j in range(1, 8):
        build_pair(j)
        if j == 2:
            # diag[p, s] = (s == p), exact in bf16 (for the blend matmul)
            nc.vector.tensor_scalar(
                diag[:], iota[:], iotac[:], None, mybir.AluOpType.is_equal
            ).then_inc(sDg)
    nc.vector.tensor_scalar(
        ohl[:], iota[:], ysl(15), None, mybir.AluOpType.is_equal
    ).then_inc(sOH)
    for j in range(8, 16):
        build_pair(j)

    # --- tensor: HAM warm-up.  The PE clock-gate opens only after
    # ~3.4us of sustained matmul activity; the PE would otherwise sit
    # idle until the first tile lands and then run the first ~4 pairs at
    # half clock.  Burn the wait on dummy matmuls over a zeroed tile.
    nc.tensor.wait_ge(sZ, 1)
    for _ in range(6):
        nc.tensor.matmul(
            ps_warm[:], lhsT=dummy[:, 0:128], rhs=dummy[:, 128:640],
            start=True, stop=True,
        )

    # --- tensor: the accumulation stream.  Groups 0..8 are half A
    # (kt0..15) into psA, stopping mid-stream; groups 9..16 are half B
    # (kt16..31) into ps.  Each half gets the full-width diagonal blend
    # (the other half's rows land in unused partitions). ---
    for g in range(ng):
        nc.tensor.wait_ge(sOH, g + 1)
        se = 16 if g == 8 else (g if g <= 7 else g - 1)
        nc.tensor.wait_ge(sE[se], 16)
        half_A = g <= 8
        bank = psA if half_A else ps
        for q, (lo, n) in enumerate(CHUNKS):
            if g == 0:
                lhsT, rhs, kwargs = oh0, t0[:, lo : lo + n], {}
            elif g == 8:
                lhsT, rhs, kwargs = ohl, t15[:, 0, lo : lo + n], {}
            else:
                j = g if g <= 7 else g - 1
                lhsT, rhs, kwargs = (
                    ohp[j - 1],
                    embt[j][:, :, lo : lo + n],
                    {"perf_mode": mybir.MatmulPerfMode.DoubleRow},
                )
            m = nc.tensor.matmul(
                bank[q][:],
                lhsT=lhsT[:],
                rhs=rhs,
                start=(g == 0 or g == 9),
                stop=(g == 8 or g == ng - 1),
                **kwargs,
            )
            if g == 8:
                m.then_inc(sMA[q])
            elif g == ng - 1:
                m.then_inc(sM[q])
        if g == 5 or g == 12:
            # blend: psum += diag @ qcent (exact diagonal, bf16) -- when
            # qcent has long landed and the PE has slack
            if g == 5:
                nc.tensor.wait_ge(sDg, 1)
                nc.tensor.wait_ge(sC, 16)
            nc.tensor.matmul(
                bank[0][:], lhsT=diag[:], rhs=centb[:, 0:512],
                start=False, stop=False,
            )
            nc.tensor.matmul(
                bank[1][:], lhsT=diag[:], rhs=centb[:, 512:896],
                start=False, stop=False,
            )
            nc.tensor.matmul(
                bank[2][:], lhsT=diag[:], rhs=centb[:, 896:D],
                start=False, stop=False,
            )

    # --- epilogue A (mid-stream): scale psA partitions 0:64 and ship
    # them while half B still streams ---
    nc.vector.wait_ge(sMA[0], 1)
    nc.vector.tensor_scalar(
        outb[0:64, 0:512], psA[0][0:64, 0:512],
        centb[0:64, D : D + 2].bitcast(_F32), None, mybir.AluOpType.mult,
    )
    nc.vector.wait_ge(sMA[2], 1)
    nc.vector.tensor_scalar(
        outb[0:64, 896:D], psA[2][0:64, 0:128],
        centb[0:64, D : D + 2].bitcast(_F32), None, mybir.AluOpType.mult,
    ).then_inc(sVA)
    nc.sync.wait_ge(sVA, 1)
    nc.sync.dma_start(out=out[0:64, 0:512], in_=outb[0:64, 0:512]).then_inc(
        sOutB, 16
    )
    nc.sync.dma_start(out=out[0:64, 896:D], in_=outb[0:64, 896:D]).then_inc(
        sOutB, 16
    )
    nc.scalar.wait_ge(sMA[1], 1)
    nc.scalar.mul(
        outb[0:64, 512:896], psA[1][0:64, 0:384],
        centb[0:64, D : D + 2].bitcast(_F32),
    ).then_inc(sAA)
    nc.scalar.wait_ge(sAA, 1)
    nc.scalar.dma_start(
        out=out[0:64, 512:896], in_=outb[0:64, 512:896]
    ).then_inc(sOutA, 16)

    # --- epilogue B (tail): scale ps partitions 64:128.  DVE takes the
    # big chunk (stops first) plus the small one, ACT the middle chunk.
    scaleB = centb[64:128, D : D + 2].bitcast(_F32)
    nc.vector.wait_ge(sM[0], 1)
    nc.vector.tensor_scalar(
        outb[64:128, 0:512], ps[0][64:128, 0:512], scaleB, None,
        mybir.AluOpType.mult,
    ).then_inc(sV1)
    nc.sync.wait_ge(sV1, 1)
    nc.sync.dma_start(
        out=out[64:128, 0:512], in_=outb[64:128, 0:512]
    ).then_inc(sOutB, 16)

    nc.scalar.wait_ge(sM[1], 1)
    nc.scalar.mul(outb[64:128, 512:896], ps[1][64:128, 0:384], scaleB).then_inc(
        sA0
    )
    nc.scalar.wait_ge(sA0, 1)
    nc.scalar.dma_start(
        out=out[64:128, 512:896], in_=outb[64:128, 512:896]
    ).then_inc(sOutA, 16)

    nc.vector.wait_ge(sM[2], 1)
    nc.vector.tensor_scalar(
        outb[64:128, 896:D], ps[2][64:128, 0:128], scaleB, None,
        mybir.AluOpType.mult,
    ).then_inc(sV2)
    nc.sync.wait_ge(sV2, 1)
    nc.sync.dma_start(
        out=out[64:128, 896:D], in_=outb[64:128, 896:D]
    ).then_inc(sOutB, 16)

    # completion: the issuing engines wait for their output DMAs, then
    # everyone meets at the final barrier
    nc.sync.wait_ge(sOutB, 64)
    nc.scalar.wait_ge(sOutA, 32)
    nc.all_engine_barrier()

    nc.compile()
    return nc


def get_nc(cap: int = 4096):
    if cap not in _CACHE:
        _CACHE[cap] = _build(cap)
    return _CACHE[cap]


def _refine(groups, sums, counts, target):
    """2-opt repair: swap classes between the max bin and any other bin
    whenever it strictly lowers max(pair); stop at max <= target."""
    for _ in range(6000):
        hi = int(np.argmax(sums))
        if sums[hi] <= target:
            return True
        best = None  # (new_pair_max, ci, cj, b, d)
        for b in range(N_CORES):
            if b == hi:
                continue
            for ci in groups[hi]:
                for cj in groups[b]:
                    d = int(counts[ci]) - int(counts[cj])
                    if d <= 0:
                        continue
                    m = max(sums[hi] - d, sums[b] + d)
                    if m < sums[hi] and (best is None or m < best[0]):
                        best = (m, ci, cj, b, d)
        if best is None:
            return False
        _m, ci, cj, b, d = best
        groups[hi].remove(ci)
        groups[b].remove(cj)
        groups[hi].append(cj)
        groups[b].append(ci)
        sums[hi] -= d
        sums[b] += d
    return bool(np.max(sums) <= target)


def _partition_classes(counts: np.ndarray):
    """Split classes into N_CORES groups, <=128 classes each, minimizing
    the max total sample count. LPT greedy + 2-opt repair, with a few
    deterministic randomized restarts to reach a perfect equipartition."""
    target = int(np.ceil(counts.sum() / N_CORES))
    order = np.argsort(-counts, kind="stable")
    best_groups, best_sums = None, None
    for seed in range(8):
        rng = np.random.default_rng(seed)
        groups = [[] for _ in range(N_CORES)]
        sums = np.zeros(N_CORES, dtype=np.int64)
        for c in order:
            cand = np.argsort(
                sums + (rng.integers(0, 2, N_CORES) if seed else 0),
                kind="stable",
            )
            for b in cand:
                if len(groups[b]) < P:
                    groups[b].append(int(c))
                    sums[b] += counts[c]
                    break
        ok = _refine(groups, sums, counts, target)
        if best_sums is None or sums.max() < best_sums.max():
            best_groups, best_sums = groups, sums
        if ok:
            break
    return best_groups, best_sums


def make_in_maps(embed: np.ndarray, y: np.ndarray, centroid: np.ndarray):
    fp8_np = mybir.dt.np(_FP8)
    bf16_np = mybir.dt.np(_BF16)
    embed8 = np.ascontiguousarray(embed, dtype=np.float32).astype(fp8_np)
    y = np.asarray(y).astype(np.int64)
    centroid = np.asarray(centroid, dtype=np.float32)
    counts = np.bincount(y, minlength=C)

    groups, sums = _partition_classes(counts)
    cap = max(4096, int(np.ceil(sums.max() / 256.0)) * 256)

    # class -> (core, slot) map
    core_of = np.full(C, -1, dtype=np.int64)
    slot_of = np.full(C, -1, dtype=np.int64)
    for i, g in enumerate(groups):
        for s, cls in enumerate(g):
            core_of[cls] = i
            slot_of[cls] = s

    kt = cap // P
    in_maps = []
    meta = []
    for i in range(N_CORES):
        rows = np.nonzero(core_of[y] == i)[0]
        n = rows.shape[0]
        emb_pad = np.zeros((cap, W), dtype=fp8_np)
        emb_pad[:n, :] = embed8[rows]
        # labels are stored as slot+256 (pads stay at 1.0, matching
        # nothing, and the on-device iota uses base=256): every byte of
        # the f32 encoding stays clear of fp8 inf/NaN bit patterns, so
        # the embedded-label bytes are benign under any fp8 view
        ys = np.full(cap, 1.0, dtype=np.float32)
        ys[:n] = slot_of[y[rows]].astype(np.float32) + 256.0
        # ysb[p, k] = slot label of padded row k*128+p, in f32
        ysb = np.ascontiguousarray(ys.reshape(kt, P).T)  # [P, kt] f32
        # emb8[p, k, :] = emb_pad[k*128 + p, :]
        emb8 = emb_pad.reshape(kt, P, W).transpose(1, 0, 2)
        # first tile: k-tile 0 + all kt f32 labels in the tail
        emb0 = np.zeros((P, W0), dtype=fp8_np)
        emb0[:, :W] = emb8[:, 0, :]
        emb0[:, W : W + 4 * kt] = ysb.view(np.uint8).view(fp8_np)
        # scale = 0.3/count quantized to 7 mantissa bits (low 16 bits of
        # the f32 zeroed) so its bytes are benign under the bf16 view;
        # qcent = 0.7*centroid/scale so the blend term is exact in bf16
        # regardless of the quantization.  Empty slots get scale=0.
        g = groups[i]
        slot_counts = np.zeros(P, dtype=np.float64)
        slot_counts[: len(g)] = counts[g]
        scale = np.where(
            slot_counts > 0, FACTOR / np.maximum(slot_counts, 1), 0.0
        ).astype(np.float32)
        scale = np.ascontiguousarray(scale)
        scale.view(np.uint32)[:] &= np.uint32(0xFFFF0000)
        cent_i = np.zeros((P, WC), dtype=bf16_np)
        qc = np.zeros((P, D), dtype=np.float64)
        qc[: len(g)] = (1.0 - FACTOR) * centroid[g].astype(np.float64)
        nz = scale > 0
        qc[nz] /= scale[nz, None].astype(np.float64)
        cent_i[:, :D] = qc.astype(np.float32).astype(bf16_np)
        cent_i[:, D : D + 2] = scale.view(bf16_np).reshape(P, 2)
        in_maps.append(
            {
                "emb0": emb0,
                "embr": np.ascontiguousarray(emb8[:, 1:, :]),
                "cent": cent_i,
            }
        )
        meta.append(g)
    return in_maps, meta, cap


def kernel(embed: np.ndarray, y: np.ndarray, centroid: np.ndarray) -> np.ndarray:
    in_maps, meta, cap = make_in_maps(embed, y, centroid)
    nc = get_nc(cap)
    res = run_bass_kernel_spmd(nc, in_maps, core_ids=list(range(N_CORES)))
    full = np.zeros((C, D), dtype=np.float32)
    for i in range(N_CORES):
        g = meta[i]
        full[g] = res.results[i]["out"][: len(g)].astype(np.float32)
    return full
